# revision 57
# baseline (speedup 1.0000x reference)
"""Multi-head attention (B=4, S=2048, D=1024, H=16, DH=64) on 8 TRN2 cores.

Sharding: core c -> (batch b = c//2, head-group g = c%2 of 8 heads).
Each core computes its batch's attention for its 8 heads plus the partial
W_O projection; the host sums the two partial outputs per batch (the
"all-reduce after W_O" done at unshard time).

v2 design (all-bf16 dataflow, attnV reorientation, xbar/PE transposes):
  - Q^T/K^T per head pair packed (128, S) bf16; V natural (s_k, h*65) bf16
    with a ones column per head.
  - scores^T (s_k chunk 128, s_q 1024) per head via single matmuls (K=64);
    exp on ACT -> bf16 P tiles [128, 1024] (s_q in 2 halves of 1024).
  - attnV REORIENTED: P chunk is the STATIONARY (M=128 s_q), V_aug [s_k,65]
    the moving -> psum [128 s_q, 65] accumulated over 16 s_k chunks; col 64
    = softmax denominator. 65 moving rows per (sq,kc) instead of 2048 for
    the V-stationary orientation: attnV drops 262k -> 133k PE cycles.
  - normalize on DVE (tensor_scalar mult by per-partition reciprocal) into
    a PAIR-STACKED onat tile (even head cols 0:64, odd 64:128 per chunk);
    each [128,128] chunk then transposes straight into otpair: via the DMA
    XBAR (exact 128x128 transpose, 8x16-tiles at 14ns) in P1-P3, via PE
    identity-transpose in P4 (keeps HWDGE free for the output DMAs).
  - out-projection: K=128 pair-stacked O^T x Wo into psum [128,512] groups.
    Half-0's groups are woven into half-1's head loop; half-1 accumulates
    pairs 0-2 into SBUF (bf16) as their otpair bands land, and the tail is
    only pair 3's matmul + an identity-matmul accumulation of acc.
  - 4-phase head interleave (H0 p01, H1 p01, H0 p23, H1 p23) spreads the
    Q/K/V projections so every phase tracks ACT's exp pace; ACT is the
    critical resource at ~267us busy, PE at ~280us.

v3 (311.4us -> 308.3us): wall ~= first_exp + ACT_busy + P1-starve + tail.
  - startup: input DMAs are BUS-bandwidth-bound (~0.73us/[128,1024], bus
    shared by all queues); critical 2.5MB (wq0,wk0,x H0) split sync/SWDGE
    even/odd, wv host-packed to one [128,4096] DMA, and the three upfront
    psum groups are dc-interleaved so every chunk is consumed on arrival.
    First exp 15.7 -> ~14us.
  - weave items whose deps land late must sit BEHIND ready work: the PE
    wait-queue is 4-deep and head-of-line blocks (v-groups stalled the
    first scores by 2.5us when placed early).
  - p0 outproj partials moved from P4 (PE-bound) into P3-h7's starved
    slots (deps complete since P2); P2-h1 (empty) took two of h3's
    groups; h1's 1.7us qkt quanta split into 0.85us halves (psS-ring
    jitter); pre-emit 2 scores BEFORE attnV(q0) (ACT idled through q0).
  - tail: fin groups ride post-attnV hooks + per-normalize-slice hooks of
    h7; combined [128,1024] out-DMAs alternate sync/SWDGE generators
    (shared-bus aware), last one on the faster sync path. PE ends 303.2,
    flush 5.1us.

All DMA-written SBUF regions are single-assignment.
"""

import sys

if "/opt/trn_rl_repo" not in sys.path:
    sys.path.insert(0, "/opt/trn_rl_repo")

import numpy as np
import ml_dtypes

import concourse.bass as bass
import concourse.tile as tile
from concourse import bacc
from concourse import mybir
from concourse import bass_utils

B, S, D, H, DH = 4, 2048, 1024, 16, 64
HL = 8              # heads per core
NCORES = 8
F32 = mybir.dt.float32
BF16 = mybir.dt.bfloat16
EXP = mybir.ActivationFunctionType.Exp

NDC = D // 128      # 8 d-chunks of 128
NKC = S // 128      # 16 s_k chunks of 128
HW = 1024           # s_q half width
NSQH = HW // 128    # 8 s_q slices per half


def _kernel_body(tc):
    nc = tc.nc
    xT = nc.dram_tensor("xT", (D, S), BF16, kind="ExternalInput").ap()
    # i-major pair layout: row p*128+i holds [dc, c] contiguous (2KB rows)
    wqp = nc.dram_tensor("wqp", (4 * 128, NDC * 128), BF16, kind="ExternalInput").ap()
    wkp = nc.dram_tensor("wkp", (4 * 128, NDC * 128), BF16, kind="ExternalInput").ap()
    # wv pre-packed on host to the SBUF layout: one [128, 8*512] DMA
    wv = nc.dram_tensor("wv", (128, NDC * HL * DH), BF16, kind="ExternalInput").ap()
    wo = nc.dram_tensor("wo", (HL * DH, D), BF16, kind="ExternalInput").ap()
    ident = nc.dram_tensor("ident", (128, 128), BF16, kind="ExternalInput").ap()
    # bf16 output: halves the DMA-bound output stream (esp. the tail);
    # the host upconverts and sums the two partials in f32.
    out = nc.dram_tensor("out", (S, D), BF16, kind="ExternalOutput").ap()

    with tc.tile_pool(name="persist", bufs=1) as persist, \
         tc.tile_pool(name="ptp", bufs=24) as ptp, \
         tc.tile_pool(name="onp", bufs=3) as onp, \
         tc.tile_pool(name="rrp", bufs=4) as rrp, \
         tc.tile_pool(name="accp", bufs=1) as accp, \
         tc.tile_pool(name="psS", bufs=2, space="PSUM") as psS, \
         tc.tile_pool(name="psV", bufs=2, space="PSUM") as psV:

        # ---- persistent SBUF ----
        qt = [persist.tile([128, S], BF16, name=f"qt{p}", tag=f"qt{p}") for p in range(4)]
        kt = [persist.tile([128, S], BF16, name=f"kt{p}", tag=f"kt{p}") for p in range(4)]
        vv = [persist.tile([128, HL * 65], BF16, name=f"v{sc}", tag=f"v{sc}") for sc in range(NKC)]
        wo_t = [persist.tile([128, D], BF16, name=f"wo{p}", tag=f"wo{p}") for p in range(4)]
        otpair = [persist.tile([128, S], BF16, name=f"otp{p}", tag=f"otp{p}") for p in range(4)]
        idt = persist.tile([128, 128], BF16, name="idt", tag="idt")

        # ---------------- head-loop helpers ----------------
        def scores_tile(Hh, h, kc):
            key = (Hh, h, kc)
            if key in pts_cache:
                return pts_cache.pop(key)
            p, rh = h // 2, h % 2
            rsl = slice(rh * 64, (rh + 1) * 64)
            ps = psS.tile([128, HW], F32, name=f"pss_{Hh}_{h}_{kc}", tag="pss")
            for half in range(2):
                nc.tensor.matmul(ps[:, half * 512:(half + 1) * 512],
                                 kt[p][rsl, kc * 128:(kc + 1) * 128],
                                 qt[p][rsl, Hh * HW + half * 512: Hh * HW + (half + 1) * 512],
                                 start=True, stop=True)
            pe = ptp.tile([128, HW], BF16, name=f"pt_{Hh}_{h}_{kc}", tag="pt")
            nc.scalar.activation(pe, ps, EXP, scale=0.125)
            return pe

        def attnv_quarter(Hh, h, q, pts):
            # psum [128 s_q, 4*65]; col s*65+64 = denominator
            # one accumulation group for the whole 2KB zero region: start
            # zeroes the full bank, every other matmul accumulates
            pv = psV.tile([128, 4 * 65], F32, name=f"psv_{Hh}_{h}_{q}", tag="psv")
            for kc in range(NKC):
                for s in range(4):
                    nc.tensor.matmul(pv[:, s * 65:(s + 1) * 65],
                                     pts[kc][:, (q * 4 + s) * 128:(q * 4 + s + 1) * 128],
                                     vv[kc][:, h * 65:(h + 1) * 65],
                                     start=(kc == 0 and s == 0),
                                     stop=(kc == NKC - 1 and s == 3))
            return pv

        def normalize_quarter(Hh, h, q, pv, onat, post_slice=None):
            # write into the PAIR-STACKED onat: sq-chunk c's 128 cols are
            # [even-head dh 0:64 | odd-head dh 64:128]. For odd heads the
            # chunk's xbar transpose fires right after its normalize slice
            # so the pair-tail chain stays short. post_slice: {c: closure}
            # emitted right after slice c's transpose+copy (the last head's
            # fin groups pipeline into the tail this way).
            p, rh = h // 2, h % 2
            rr = rrp.tile([128, 4], F32, name=f"rr_{Hh}_{h}_{q}", tag="rr")
            nc.vector.reciprocal(rr, pv.rearrange("p (s c) -> p s c", c=65)[:, :, 64])
            for s in range(4):
                c = q * 4 + s
                nc.vector.tensor_scalar_mul(
                    onat[:, c * 128 + rh * 64: c * 128 + rh * 64 + 64],
                    pv[:, s * 65: s * 65 + 64],
                    rr[:, s:s + 1])
                if rh == 1:
                    pool = pe_transpose_pool[0]
                    if pool is None:
                        nc.sync.dma_start_transpose(
                            out=otpair[p][:, Hh * HW + c * 128: Hh * HW + (c + 1) * 128],
                            in_=onat[:, c * 128:(c + 1) * 128])
                    else:
                        # P4: PE transpose via identity into a psF-tag slot
                        # (same 2KB byte size) -> short tail chain, HWDGE
                        # stays free for the out-DMAs
                        tp = pool.tile([128, 1024], BF16, name=f"tp_{Hh}_{h}_{c}",
                                       tag="pf")
                        nc.tensor.transpose(tp[:, 0:128],
                                            onat[:, c * 128:(c + 1) * 128], idt)
                        nc.vector.tensor_copy(
                            otpair[p][:, Hh * HW + c * 128: Hh * HW + (c + 1) * 128],
                            tp[:, 0:128])
                if post_slice is not None and c in post_slice:
                    post_slice[c]()

        onats = {}
        pe_transpose_pool = [None]   # set to psF pool during P4
        pts_cache = {}               # (Hh, h, kc) -> pre-emitted P tile

        # H1-half outproj partial accumulators: pair p's [128,512] matmul
        # lands in `pool` (ppp during P3, psF during P4), then DVE folds it
        # into the bf16 acc tile. p0 partials run in P3-h7's starved PE slots.
        acc = [accp.tile([128, 512], BF16, name=f"acc{g}", tag=f"acc{g}")
               for g in range(16)]

        def partial_h1(p, q16, dcol, pool):
            def emit():
                g = q16 * 2 + dcol
                pf = pool.tile([128, 512], F32, name=f"pf1_{p}_{g}", tag=pool._qkt_tag)
                nc.tensor.matmul(pf, otpair[p][:, HW + q16 * 128: HW + (q16 + 1) * 128],
                                 wo_t[p][:, dcol * 512:(dcol + 1) * 512],
                                 start=True, stop=True)
                if p == 0:
                    nc.vector.tensor_copy(acc[g], pf)
                else:
                    nc.vector.tensor_add(acc[g], acc[g], pf)
            return emit

        def emit_head(Hh, h, weave, start_pop=0, nxt=None, pop_every=1,
                      post_q0=None, post_slice_q1=None):
            """weave: closures popped between scores tiles; any leftovers are
            flushed BEFORE attnV so a weave item can never sit behind its
            consumer in the in-order PE stream. start_pop delays the first
            pop. nxt=(Hh2,h2): pre-emit that head's first scores between this
            head's attnV quarters so ACT never bubbles at head boundaries.
            post_q0: closures after attnV(q1) (overlap normalize q1's DVE);
            post_slice_q1: {c: [closures]} per normalize-q1 slice."""
            p, rh = h // 2, h % 2
            pts = []
            for kc in range(NKC):
                pts.append(scores_tile(Hh, h, kc))
                if weave and kc >= start_pop and (kc - start_pop) % pop_every == 0:
                    f = weave.pop(0)
                    if f is not None:    # None = deliberate empty slot
                        f()
            while weave:
                f = weave.pop(0)
                if f is not None:
                    f()
            if rh == 0:
                onats[(Hh, p)] = onp.tile([128, HW], BF16, name=f"on_{Hh}_{p}", tag="on")
            onat = onats[(Hh, p)]
            # pre-emit 2 of the next head's scores BEFORE attnV(q0) so ACT
            # chews them through q0 (not just q1), +1 between the quarters
            if nxt is not None:
                for kc2 in range(2):
                    pts_cache[(nxt[0], nxt[1], kc2)] = scores_tile(nxt[0], nxt[1], kc2)
            for q in range(2):
                pv = attnv_quarter(Hh, h, q, pts)
                if q == 0 and nxt is not None:
                    pts_cache[(nxt[0], nxt[1], 2)] = scores_tile(nxt[0], nxt[1], 2)
                if q == 1 and post_q0 is not None:
                    for f in post_q0:
                        f()
                normalize_quarter(Hh, h, q, pv, onat,
                                  post_slice=(post_slice_q1 if q == 1 else None))

        with tc.tile_pool(name="xw", bufs=1) as xw:
            xt = [xw.tile([128, S], BF16, name=f"xt{dc}", tag=f"xt{dc}") for dc in range(NDC)]
            wqt = [xw.tile([128, NDC * 128], BF16, name=f"wq{p}", tag=f"wq{p}") for p in range(4)]
            wkt = [xw.tile([128, NDC * 128], BF16, name=f"wk{p}", tag=f"wk{p}") for p in range(4)]
            wvt = xw.tile([128, NDC * HL * DH], BF16, name="wvt", tag="wvt")

            # --- input DMAs: critical path on sync (HWDGE), rest on gpsimd
            # (SWDGE): the two generator paths run in parallel.
            def load_wpair(wt, src, p, queue):
                queue.dma_start(out=wt[p], in_=src[p * 128:(p + 1) * 128, :])

            # startup-critical path: the HWDGE generator paces input arrival
            # (~0.7-0.85us per DMA regardless of width), so (a) load xt H0
            # halves as FULL [128,1024] DMAs -- the q01/kt-cc1 groups then
            # need no extra transfers -- and (b) fan the loads across THREE
            # generators: SP queue (wq0, xt0-3, wk0), ACT queue (xt4-7; ACT
            # is idle until its first exp ~8us in), SWDGE (wv, then xt H1
            # halves, consumed only ~15us+ in).
            # Input loads are DMA-BANDWIDTH-bound (~0.73us per [128,1024] on a
            # bus shared by ALL queues), so strict priority order matters:
            # wq0, xt0[0:HW], wk0, xt1-7[0:HW] feed the first three psum
            # groups; everything else follows. SWDGE gens execute ON the Pool
            # engine, so memsets go first there and wv gens fire ~9us in
            # (wvt needed ~11us by the first v_groups).
            # Critical stream saturates the shared DMA bus from BOTH
            # generators: even xt chunks on the sync HWDGE, odd on the
            # gpsimd SWDGE; wv (one packed DMA) right behind, then the x
            # H1 halves, then weight pairs / wo.
            load_wpair(wqt, wqp, 0, nc.sync)
            nc.sync.dma_start(out=xt[0][:, 0:HW], in_=xT[0:128, 0:HW])
            load_wpair(wkt, wkp, 0, nc.sync)
            for dc in range(2, NDC, 2):
                nc.sync.dma_start(out=xt[dc][:, 0:HW], in_=xT[dc * 128:(dc + 1) * 128, 0:HW])
            for dc in range(0, NDC, 2):
                nc.sync.dma_start(out=xt[dc][:, HW:S], in_=xT[dc * 128:(dc + 1) * 128, HW:S])
            for dc in range(1, NDC, 2):
                nc.gpsimd.dma_start(out=xt[dc][:, 0:HW], in_=xT[dc * 128:(dc + 1) * 128, 0:HW])
            nc.gpsimd.dma_start(out=wvt, in_=wv)
            for dc in range(1, NDC, 2):
                nc.gpsimd.dma_start(out=xt[dc][:, HW:S], in_=xT[dc * 128:(dc + 1) * 128, HW:S])
            # memsets AFTER the SWDGE gens above (Pool runs the generator)
            for sc in range(NKC):
                nc.gpsimd.memset(vv[sc], 1.0)
            # weight pairs 1-3 on sync behind the x stream (needed 20us+ in)
            for p in range(1, 4):
                load_wpair(wqt, wqp, p, nc.sync)
                load_wpair(wkt, wkp, p, nc.sync)
            nc.gpsimd.dma_start(out=idt, in_=ident)
            for p in range(4):
                nc.gpsimd.dma_start(out=wo_t[p], in_=wo[p * 128:(p + 1) * 128, :])

            # --- projection group emitters (weave quanta) ---
            def qkt_group(pool, wt, dst, p, cc):
                # psum [128 pair-dh, 512 s_q]: dst cols cc*512:(cc+1)*512
                def emit():
                    ps = pool.tile([128, 512], F32, name=f"pp{p}_{cc}", tag=pool._qkt_tag)
                    for dc in range(NDC):
                        nc.tensor.matmul(ps, wt[p][:, dc * 128:(dc + 1) * 128],
                                         xt[dc][:, cc * 512:(cc + 1) * 512],
                                         start=(dc == 0), stop=(dc == NDC - 1))
                    nc.vector.tensor_copy(dst[p][:, cc * 512:(cc + 1) * 512], ps)
                return emit

            def qkt_halves(pool, wt, dst, p, cc):
                # same group split into two 0.85us weave quanta (the 1.7us
                # whole-group quantum makes PE slots jitter past ACT's 1.04us
                # exp pace and starves the psS ring in PE-bound P1)
                state = {}

                def emit_a():
                    ps = pool.tile([128, 512], F32, name=f"pp{p}_{cc}", tag=pool._qkt_tag)
                    state["ps"] = ps
                    for dc in range(4):
                        nc.tensor.matmul(ps, wt[p][:, dc * 128:(dc + 1) * 128],
                                         xt[dc][:, cc * 512:(cc + 1) * 512],
                                         start=(dc == 0), stop=False)

                def emit_b():
                    ps = state["ps"]
                    for dc in range(4, NDC):
                        nc.tensor.matmul(ps, wt[p][:, dc * 128:(dc + 1) * 128],
                                         xt[dc][:, cc * 512:(cc + 1) * 512],
                                         start=False, stop=(dc == NDC - 1))
                    nc.vector.tensor_copy(dst[p][:, cc * 512:(cc + 1) * 512], ps)
                return [emit_a, emit_b]

            def v_group(pool, sc, hp):
                # psum [128 s_k, 128 (2 heads)] -> scatter into stride-65 slots
                def emit():
                    ps = pool.tile([128, 512], F32, name=f"pv_{sc}_{hp}", tag=pool._qkt_tag)
                    for dc in range(NDC):
                        nc.tensor.matmul(ps[:, 0:128], xt[dc][:, sc * 128:(sc + 1) * 128],
                                         wvt[:, dc * 512 + hp * 128: dc * 512 + (hp + 1) * 128],
                                         start=(dc == 0), stop=(dc == NDC - 1))
                    vsrc = ps[:, 0:128].rearrange("p (h x) -> p h x", x=64)
                    vdst = vv[sc].rearrange("p (h x) -> p h x", x=65)[:, 2 * hp:2 * hp + 2, 0:64]
                    nc.vector.tensor_copy(vdst, vsrc)
                return emit

            # 4-phase head interleave: H0 pairs 0/1, H1 pairs 0/1, H0 pairs
            # 2/3, H1 pairs 2/3. Projections spread across P1-P3 so every
            # phase's PE load tracks ACT's 16.6us/head exp pace.
            # NOTE: kt is indexed by the KEY axis (kc spans full S) so a
            # pair's kt must be fully built before its first scores; only
            # qt splits by s_q half (cc 0,1 = H0; cc 2,3 = H1).
            with tc.tile_pool(name="ppp", bufs=2, space="PSUM") as ppp:
                ppp._qkt_tag = "pp"
                # upfront: first scores(h0) needs qt p0 cc0+cc1, kt p0 cc0.
                # All THREE groups are dc-INTERLEAVED so each xt chunk is
                # consumed the moment its DMA lands; q01 borrows a psS slot
                # (scores psum is idle until the first exp) since ppp only
                # has 2 bufs.
                ps_q = ppp.tile([128, 512], F32, name="pp_q00", tag="pp")
                ps_k = ppp.tile([128, 512], F32, name="pp_k00", tag="pp")
                ps_q1 = psS.tile([128, 512], F32, name="pp_q01", tag="pss")
                for dc in range(NDC):
                    nc.tensor.matmul(ps_q, wqt[0][:, dc * 128:(dc + 1) * 128],
                                     xt[dc][:, 0:512], start=(dc == 0), stop=(dc == NDC - 1))
                    nc.tensor.matmul(ps_k, wkt[0][:, dc * 128:(dc + 1) * 128],
                                     xt[dc][:, 0:512], start=(dc == 0), stop=(dc == NDC - 1))
                    nc.tensor.matmul(ps_q1, wqt[0][:, dc * 128:(dc + 1) * 128],
                                     xt[dc][:, 512:1024], start=(dc == 0), stop=(dc == NDC - 1))
                nc.vector.tensor_copy(qt[0][:, 0:512], ps_q)
                # kt0 copy on ACT (idle before the first exp): the three
                # startup copies drain two engines wide
                nc.scalar.copy(kt[0][:, 0:512], ps_k)
                nc.vector.tensor_copy(qt[0][:, 512:1024], ps_q1)

                G = qkt_group
                # ---- P1: H0 heads 0-3 ----
                # arrival-ordered weave; items whose deps land late (v0/v1
                # wait wvt ~14us, kt0 cc2/3 wait x-H1 ~16us) must sit BEHIND
                # ready work -- the PE wait-queue is only 4 deep and
                # head-of-line blocks the stream.
                w = {0: qkt_halves(ppp, wkt, kt, 0, 1)
                        + [v_group(ppp, 0, 0), v_group(ppp, 1, 0)]
                        + qkt_halves(ppp, wkt, kt, 0, 2)
                        + qkt_halves(ppp, wkt, kt, 0, 3)
                        + [v_group(ppp, sc, 0) for sc in range(2, NKC)],
                     1: [x for pc in [(wkt, kt, 1, 0), (wkt, kt, 1, 1),
                                      (wkt, kt, 1, 2), (wkt, kt, 1, 3),
                                      (wqt, qt, 1, 0), (wqt, qt, 1, 1)]
                         for x in qkt_halves(ppp, pc[0], pc[1], pc[2], pc[3])],
                     2: [v_group(ppp, sc, 1) for sc in range(NKC)],
                     3: [x for pc in [(wqt, qt, 1, 2), (wqt, qt, 1, 3)]
                         for x in qkt_halves(ppp, pc[0], pc[1], pc[2], pc[3])]}
                for h in range(4):
                    emit_head(0, h, w[h], nxt=[(0, 1), (0, 2), (0, 3), (1, 2)][h],
                              pop_every={3: 4}.get(h, 1))

                # ---- P2: H1 heads 2,3,0,1 ----
                # h1 (previously empty) takes two of h3's groups: kt2 is still
                # fully built before h1's attnV pre-emits (0,4)'s scores
                w = {2: [G(ppp, wqt, qt, 0, 2), G(ppp, wqt, qt, 0, 3),
                         G(ppp, wkt, kt, 2, 0), G(ppp, wkt, kt, 2, 1)],
                     3: [G(ppp, wkt, kt, 2, 2), G(ppp, wqt, qt, 2, 0)],
                     0: [v_group(ppp, sc, 2) for sc in range(NKC)],
                     1: [G(ppp, wkt, kt, 2, 3), G(ppp, wqt, qt, 2, 1)]}
                nxts = {2: (1, 3), 3: (1, 0), 0: (1, 1), 1: (0, 4)}
                for h in (2, 3, 0, 1):
                    emit_head(1, h, w[h], nxt=nxts[h],
                              pop_every=(4 if h in (2, 3) else 1))

                # ---- P3: H0 heads 4-7 ----
                # h7 additionally absorbs ALL p0 outproj partials (deps done
                # since P2): its 2-item weave left PE starved against the psS
                # ring while P4 -- a PE-bound phase -- queued this same work.
                w = {4: [G(ppp, wkt, kt, 3, 0), G(ppp, wkt, kt, 3, 1),
                         G(ppp, wkt, kt, 3, 2), G(ppp, wkt, kt, 3, 3)],
                     5: [G(ppp, wqt, qt, 3, 0), G(ppp, wqt, qt, 3, 1),
                         G(ppp, wqt, qt, 2, 2), G(ppp, wqt, qt, 2, 3)],
                     6: [v_group(ppp, sc, 3) for sc in range(NKC)],
                     7: [G(ppp, wqt, qt, 3, 2), G(ppp, wqt, qt, 3, 3)]
                        + [partial_h1(0, q16, dcol, ppp)
                           for q16 in range(NSQH) for dcol in range(2)]}
                for h in range(4, 8):
                    emit_head(0, h, w[h],
                              nxt=[(0, 5), (0, 6), (0, 7), (1, 4)][h - 4],
                              pop_every={4: 4, 5: 4}.get(h, 1))

        # ---- P4: H1 heads 4-7, with half-0 outproj + half-1 partial
        # outproj (pairs accumulated in SBUF as their otpair bands land;
        # the tail is only pair 3's 16 finalize groups) ----
        with tc.tile_pool(name="stp", bufs=8) as stp, \
             tc.tile_pool(name="psF", bufs=2, space="PSUM") as psF:
            psF._qkt_tag = "pf"

            sts = {}

            def outproj_h0(q16, dcol):
                # dcol pair shares one st tile; ONE 4KB-row DMA per q16
                def emit():
                    pf = psF.tile([128, 512], F32, name=f"pf0_{q16}_{dcol}", tag="pf")
                    for p in range(4):
                        nc.tensor.matmul(
                            pf, otpair[p][:, q16 * 128:(q16 + 1) * 128],
                            wo_t[p][:, dcol * 512:(dcol + 1) * 512],
                            start=(p == 0), stop=(p == 3))
                    if dcol == 0:
                        sts[(0, q16)] = stp.tile([128, 1024], BF16, name=f"st0_{q16}", tag="st")
                    st = sts[(0, q16)]
                    nc.vector.tensor_copy(st[:, dcol * 512:(dcol + 1) * 512], pf)
                    if dcol == 1:
                        nc.sync.dma_start(
                            out=out[q16 * 128:(q16 + 1) * 128, :], in_=st)
                return emit

            def final_h1(q16, kind):
                # pair-3 matmul + identity-matmul accumulation of the bf16
                # acc (pairs 0-2): keeps the tail add off the DVE. identity
                # first: no dependency on the last transposes. kind picks the
                # psum ring ("pss": one [128,1024] slot; "psv": two [128,512]
                # slots) -- both rings are free in the tail; explicit
                # assignment avoids blocking on pv(q1)'s release.
                def emit():
                    if kind == "pss":
                        pft = psS.tile([128, HW], F32, name=f"pff_{q16}", tag="pss")
                        halves = [pft[:, 0:512], pft[:, 512:1024]]
                    else:
                        halves = [psV.tile([128, 512], F32, name=f"pff_{q16}", tag="psv"),
                                  psV.tile([128, 512], F32, name=f"pffb_{q16}", tag="psv")]
                    st = stp.tile([128, 1024], BF16, name=f"stf_{q16}", tag="st")
                    for dcol in range(2):
                        g = q16 * 2 + dcol
                        pfh = halves[dcol]
                        nc.tensor.matmul(pfh, idt, acc[g], start=True, stop=False)
                        nc.tensor.matmul(pfh, otpair[3][:, HW + q16 * 128: HW + (q16 + 1) * 128],
                                         wo_t[3][:, dcol * 512:(dcol + 1) * 512],
                                         start=False, stop=True)
                        if dcol == 0:
                            # ACT is idle post-exp: split the psum->sbuf copies
                            nc.scalar.copy(st[:, 0:512], pfh)
                        else:
                            nc.vector.tensor_copy(st[:, 512:1024], pfh)
                    # ONE combined [128,1024] DMA per q16, alternating the SP
                    # HWDGE queue with the gpsimd SWDGE queue so the tail
                    # flush drains two generators wide; the LAST group rides
                    # the faster HWDGE gen (SWDGE costs ~1us more end-to-end).
                    queue = nc.sync if (q16 % 2 == 0 or q16 == 7) else nc.gpsimd
                    queue.dma_start(
                        out=out[HW + q16 * 128: HW + (q16 + 1) * 128, :], in_=st)
                return emit

            pe_transpose_pool[0] = psF
            op0 = [outproj_h0(q16, dcol) for q16 in range(NSQH) for dcol in range(2)]
            # PE-cycle-balanced P4 weave (~6us/head = ACT's exp pace). p0
            # partials already ran in P3-h7; p2's otpair[2] H1 dep completes
            # with h5, so they sit in h6; h7 takes the op0 tail.
            def mix(parts, ops):
                lst, k = [], 0
                for i, x in enumerate(parts):
                    lst.append(x)
                    if (i + 1) % 6 == 0 and k < len(ops):
                        lst.append(ops[k]); k += 1
                return lst + ops[k:]
            w = {4: mix([partial_h1(1, q16, dcol, psF)
                         for q16 in range(NSQH) for dcol in range(2)], op0[0:2]),
                 5: op0[2:7],
                 6: mix([partial_h1(2, q16, dcol, psF)
                         for q16 in range(NSQH) for dcol in range(2)], op0[7:10]),
                 7: op0[10:16]}
            # fin pipeline: fin0-2 right after attnV(q1) (their bands landed
            # in quarter 0); fin3-7 woven per normalize-q1 slice so only the
            # last group trails the final transpose. psum kinds chosen so no
            # fin blocks on pv(q1)'s release (psV slot B frees only after
            # normalize q1's muls).
            fin = {0: final_h1(0, "pss"), 1: final_h1(1, "psv"),
                   2: final_h1(2, "pss"), 3: final_h1(3, "pss"),
                   4: final_h1(4, "pss"), 5: final_h1(5, "psv"),
                   6: final_h1(6, "pss"), 7: final_h1(7, "psv")}

            def two(a, b):
                def emit():
                    a(); b()
                return emit
            for h in range(4, 8):
                emit_head(1, h, w[h], start_pop=(2 if h == 6 else 0),
                          nxt=([(1, 5), (1, 6), (1, 7)] + [None])[h - 4],
                          pop_every=(3 if h in (5, 7) else 1),
                          post_q0=([fin[0], fin[1], fin[2]] if h == 7 else None),
                          post_slice_q1=({4: fin[3], 5: fin[4],
                                          6: fin[5], 7: two(fin[6], fin[7])}
                                         if h == 7 else None))


_NC_CACHE = None


def _get_nc():
    global _NC_CACHE
    if _NC_CACHE is None:
        nc = bacc.Bacc("TRN2", target_bir_lowering=False, debug=False)
        with tile.TileContext(nc) as tc:
            _kernel_body(tc)
        nc.compile()
        _NC_CACHE = nc
    return _NC_CACHE


def _out_perm():
    """pair-stacked onat + 128x128 xbar transposes leave rows unpermuted."""
    return np.arange(S)


def _shard_inputs(x, Wq, Wk, Wv, Wo):
    bf = ml_dtypes.bfloat16
    in_maps = []
    for c in range(NCORES):
        b, g = c // 2, c % 2
        xT = np.ascontiguousarray(x[b].T).astype(bf)
        sl = slice(HL * g, HL * (g + 1))
        wq_s = Wq[sl].transpose(1, 0, 2).reshape(D, HL * DH)
        wk_s = Wk[sl].transpose(1, 0, 2).reshape(D, HL * DH)
        # per-pair blocks: (4, D, 128) flattened to (4D, 128)
        # (4, 128 i, 8 dc, 128 c): row p*128+i holds all dc chunks' cols
        wqp = np.ascontiguousarray(
            wq_s.reshape(8, 128, 4, 128).transpose(2, 1, 0, 3)).reshape(4 * 128, 8 * 128).astype(bf)
        wkp = np.ascontiguousarray(
            wk_s.reshape(8, 128, 4, 128).transpose(2, 1, 0, 3)).reshape(4 * 128, 8 * 128).astype(bf)
        # packed to SBUF layout: partition p, col dc*512+c = Wv[d=dc*128+p, c]
        wv_s = np.ascontiguousarray(
            Wv[sl].transpose(1, 0, 2).reshape(8, 128, HL * DH)
            .transpose(1, 0, 2).reshape(128, 8 * HL * DH)).astype(bf)
        wo_s = np.ascontiguousarray(Wo[HL * DH * g: HL * DH * (g + 1), :]).astype(bf)
        in_maps.append({"xT": xT, "wqp": wqp, "wkp": wkp, "wv": wv_s, "wo": wo_s,
                        "ident": np.eye(128, dtype=np.float32).astype(bf)})
    return in_maps


def kernel(**inputs):
    x = np.asarray(inputs["x"], dtype=np.float32)
    Wq = np.asarray(inputs["Wq"], dtype=np.float32)
    Wk = np.asarray(inputs["Wk"], dtype=np.float32)
    Wv = np.asarray(inputs["Wv"], dtype=np.float32)
    Wo = np.asarray(inputs["Wo"], dtype=np.float32)

    nc = _get_nc()
    in_maps = _shard_inputs(x, Wq, Wk, Wv, Wo)
    res = None
    for attempt in range(3):
        try:
            res = bass_utils.run_bass_kernel_spmd(nc, in_maps, core_ids=list(range(NCORES)))
            break
        except Exception:
            # transient axon/NRT device errors recover on retry
            if attempt == 2:
                raise
            import time
            time.sleep(20)
    outs = [np.asarray(res.results[c]["out"], dtype=np.float32) for c in range(NCORES)]
    full = np.stack([outs[2 * b] + outs[2 * b + 1] for b in range(B)], axis=0)
    return full.astype(np.float32)

